# revision 17
# baseline (speedup 1.0000x reference)
"""Trainium2 Bass kernel for the AttentionTimeSeriesTransformer problem.

Strategy:
  - Data-parallel over batch: 32 batches -> 4 per core x 8 cores. No collectives.
  - Activations kept feature-major in SBUF: xT [D=512, T=2048] (T = 4 batches x 512 seq).
  - All matmuls in float32r (FP22 mantissa truncation, full PE rate, N=512 moving dim).
  - Attention computed as scoresT [key, query] so softmax's sum-reduction lands on the
    partition dim where it is done by matmul; numerical stabilization via key-centering
    (subtracting the per-(b,h) key mean from K folds the per-query score mean into the
    scores matmul - softmax is shift-invariant so the result is exact).
  - Softmax denominator: a ones-column appended to V's stationary operand makes the AV
    matmul emit sum(exp) as row 64; reciprocal + gpsimd partition_broadcast + multiply.
  - LayerNorm in feature-major: sums/sumsq via ones-vector matmuls, row stats on 1
    partition, gpsimd broadcast, fused DVE normalize ops.
"""

import math

import numpy as np

# Problem constants (hardcoded per task contract).
D = 512
H = 8
L = 6
FF = 2048
B = 32
S = 512
IN = 64
OUT = 1
HD = D // H  # 64
EPS = 1e-5
NCORES = 8
BL = B // NCORES  # 4 batches per core
T = BL * S  # 2048 tokens per core
SQRT_D = math.sqrt(D)
KT = D // 128  # 4 k-tiles over feature dim
FT = FF // 128  # 16 tiles over ff dim

# cols tensor column layout (per layer, [128, 64] f32):
#  0-11 : in_proj_b slices (m = 0..11)
# 12-15 : out_b slices (e = 0..3)
# 16-31 : ff_b1 slices (f = 0..15)
# 32-35 : ff_b2 slices (e = 0..3)
# 36-39 : ln1_g slices; 40-43: ln1_b; 44-47: ln2_g; 48-51: ln2_b
# 52    : ones (for stat matmuls)
# 53-56 : inp_b slices * sqrt(D) (used by layer-0 input projection only)
C_QKVB = 0
C_OUTB = 12
C_FFB1 = 16
C_FFB2 = 32
C_LN1G = 36
C_LN1B = 40
C_LN2G = 44
C_LN2B = 48
C_ONES = 52
C_INPB = 53
C_SOFTOFF = 57  # -(per-layer softmax stabilizer offset), applied inside exp bias

# Per-layer constant subtracted from scaled, key-centered scores before exp.
# Softmax is shift-invariant so this is mathematically exact; it only guards
# fp32 exp range. Layer 0 has huge pre-LN activations (std(scaled scores)~22,
# max ~112 > 88); later layers are post-LN and tiny.
SOFT_OFF = [76.0, 0.0, 0.0, 0.0, 0.0, 0.0]


def _pos_encoding_T():
    """pe.T [D, S] in float32, matching reference._pos_encoding."""
    position = np.arange(S, dtype=np.float32)[:, None]
    div_term = np.exp(
        np.arange(0, D, 2, dtype=np.float32) * np.float32(-math.log(10000.0) / D)
    )
    ang = position * div_term  # [S, D/2]
    pe = np.zeros((S, D), dtype=np.float32)
    pe[:, 0::2] = np.sin(ang)
    pe[:, 1::2] = np.cos(ang)
    return np.ascontiguousarray(pe.T)


def build_program():
    """Build the Bass/Tile program (same SPMD program for every core)."""
    import concourse.bass as bass
    import concourse.mybir as mybir
    import concourse.tile as tile
    from concourse import bacc

    f32 = mybir.dt.float32
    f32r = mybir.dt.float32r
    bf16 = mybir.dt.bfloat16
    OPT = mybir.AluOpType
    AF = mybir.ActivationFunctionType
    AX = mybir.AxisListType.X

    def r(ap):
        return ap.bitcast(f32r)

    nc = bacc.Bacc("TRN2", target_bir_lowering=False, debug=False)

    dsrcT = nc.dram_tensor("srcT", [IN, T], f32, kind="ExternalInput")
    dinpwT = nc.dram_tensor("inpwT", [IN, D], f32, kind="ExternalInput")
    dpeT = nc.dram_tensor("peT", [D, S], f32, kind="ExternalInput")
    dwqkvT = nc.dram_tensor("wqkvT", [L, D, 3 * D], f32, kind="ExternalInput")
    dwoT = nc.dram_tensor("woT", [L, D, D], f32, kind="ExternalInput")
    dw1T = nc.dram_tensor("w1T", [L, D, FF], f32, kind="ExternalInput")
    dw2T = nc.dram_tensor("w2T", [L, FF, D], f32, kind="ExternalInput")
    dcols = nc.dram_tensor("cols", [L, 128, 64], f32, kind="ExternalInput")
    dhw1T = nc.dram_tensor("hw1T", [D, FF], f32, kind="ExternalInput")
    dhw2c = nc.dram_tensor("hw2c", [128, FT], f32, kind="ExternalInput")
    dhb1c = nc.dram_tensor("hb1c", [128, FT], f32, kind="ExternalInput")
    dhb2 = nc.dram_tensor("hb2", [1, 1], f32, kind="ExternalInput")
    dout = nc.dram_tensor("out", [BL, OUT], f32, kind="ExternalOutput")

    with tile.TileContext(nc) as tc:
        import contextlib

        ctx = contextlib.ExitStack()
        with ctx:
            px = ctx.enter_context(tc.tile_pool(name="px", bufs=17))
            pqk = ctx.enter_context(tc.tile_pool(name="pqk", bufs=10))
            pv = ctx.enter_context(tc.tile_pool(name="pv", bufs=4))
            pkb = ctx.enter_context(tc.tile_pool(name="pkb", bufs=3))
            pe_ = ctx.enter_context(tc.tile_pool(name="pe", bufs=6))
            po = ctx.enter_context(tc.tile_pool(name="po", bufs=4))
            py = ctx.enter_context(tc.tile_pool(name="py", bufs=4))
            ph = ctx.enter_context(tc.tile_pool(name="ph", bufs=9))
            psq = ctx.enter_context(tc.tile_pool(name="psq", bufs=2))
            prow = ctx.enter_context(tc.tile_pool(name="prow", bufs=4))
            pbc = ctx.enter_context(tc.tile_pool(name="pbc", bufs=4))
            pwqkv = ctx.enter_context(tc.tile_pool(name="pwqkv", bufs=4))
            pwo = ctx.enter_context(tc.tile_pool(name="pwo", bufs=5))
            pw1 = ctx.enter_context(tc.tile_pool(name="pw1", bufs=5))
            pw2 = ctx.enter_context(tc.tile_pool(name="pw2", bufs=6))
            pc = ctx.enter_context(tc.tile_pool(name="pc", bufs=2))
            psmall = ctx.enter_context(tc.tile_pool(name="psmall", bufs=4))

            pmm = ctx.enter_context(tc.tile_pool(name="pmm", bufs=2, space="PSUM"))
            psc = ctx.enter_context(tc.tile_pool(name="psc", bufs=2, space="PSUM"))
            pff = ctx.enter_context(tc.tile_pool(name="pff", bufs=2, space="PSUM"))
            pav = ctx.enter_context(tc.tile_pool(name="pav", bufs=2, space="PSUM"))

            def load_layer_weights(l):
                w = {}
                w["qkv"] = []
                for k in range(KT):
                    t_ = pwqkv.tile([128, 3 * D], f32, name=f"wqkv_{l}_{k}", tag="wqkv")
                    nc.sync.dma_start(r(t_), r(dwqkvT.ap()[l, k * 128 : (k + 1) * 128, :]))
                    w["qkv"].append(t_)
                w["wo"] = []
                for k in range(KT):
                    t_ = pwo.tile([128, D], f32, name=f"wo_{l}_{k}", tag="wo")
                    nc.sync.dma_start(r(t_), r(dwoT.ap()[l, k * 128 : (k + 1) * 128, :]))
                    w["wo"].append(t_)
                w["w1"] = []
                for k in range(KT):
                    t_ = pw1.tile([128, FF], f32, name=f"w1_{l}_{k}", tag="w1")
                    nc.sync.dma_start(r(t_), r(dw1T.ap()[l, k * 128 : (k + 1) * 128, :]))
                    w["w1"].append(t_)
                cols = pc.tile([128, 64], f32, name=f"cols_{l}", tag="cols")
                nc.sync.dma_start(r(cols), r(dcols.ap()[l]))
                w["cols"] = cols
                return w

            def col(cols, i):
                return cols[:, i : i + 1]

            # ---------------- input projection -----------------
            wts = load_layer_weights(0)
            cols0 = wts["cols"]

            srcT_sb = pw1.tile([IN, T], f32, name="srcT_sb", tag="w1")
            nc.sync.dma_start(r(srcT_sb), r(dsrcT.ap()))
            inpw_sb = pwo.tile([IN, D], f32, name="inpw_sb", tag="wo")
            nc.sync.dma_start(r(inpw_sb), r(dinpwT.ap()))
            pe_sb = []
            for m in range(KT):
                t_ = pbc.tile([128, S], f32, name=f"pe_sb_{m}", tag="bc")
                nc.sync.dma_start(t_, dpeT.ap()[m * 128 : (m + 1) * 128, :])
                pe_sb.append(t_)

            x = {}
            for m in range(KT):
                for b in range(BL):
                    ps = pmm.tile([128, 512], f32, name=f"ps_in_{m}_{b}", tag="mm")
                    nc.tensor.matmul(
                        ps,
                        r(inpw_sb[:, m * 128 : (m + 1) * 128]),
                        r(srcT_sb[:, b * S : (b + 1) * S]),
                        start=True,
                        stop=True,
                    )
                    xt = px.tile([128, 512], f32, name=f"x0_{m}_{b}", tag="x")
                    nc.scalar.activation(
                        r(xt), ps, AF.Identity, bias=col(cols0, C_INPB + m), scale=SQRT_D
                    )
                    nc.vector.tensor_tensor(r(xt), xt, pe_sb[m], OPT.add)
                    x[(m, b)] = xt

            # ---------------- transformer layers -----------------
            for l in range(L):
                cols = wts["cols"]
                x_next = {}

                # attention + out_proj + LN1, pipelined per batch
                x1all = {}
                for b in range(BL):
                    # Q, K projections (feature-major): qk[m] rows = d_out slice
                    qk = []
                    for m in range(8):
                        ps = pmm.tile([128, 512], f32, name=f"ps_qk_{l}_{b}_{m}", tag="mm")
                        for k in range(KT):
                            nc.tensor.matmul(
                                ps,
                                r(wts["qkv"][k][:, m * 128 : (m + 1) * 128]),
                                r(x[(k, b)]),
                                start=(k == 0),
                                stop=(k == KT - 1),
                            )
                        qt = pqk.tile([128, 512], f32, name=f"qk_{l}_{b}_{m}", tag="qk")
                        nc.scalar.activation(
                            r(qt), ps, AF.Identity, bias=col(cols, C_QKVB + m), scale=1.0
                        )
                        qk.append(qt)

                    # V projection (token-major): v[tm] = [128 tokens, 8 heads x (64+1)]
                    vts = []
                    for tm in range(4):
                        ps = pmm.tile([128, 512], f32, name=f"ps_v_{l}_{b}_{tm}", tag="mm")
                        for k in range(KT):
                            nc.tensor.matmul(
                                ps,
                                r(x[(k, b)][:, tm * 128 : (tm + 1) * 128]),
                                r(wts["qkv"][k][:, 2 * D : 3 * D]),
                                start=(k == 0),
                                stop=(k == KT - 1),
                            )
                        vt = pv.tile([128, 8 * (HD + 1)], bf16, name=f"v_{l}_{b}_{tm}", tag="v")
                        vt_r = vt.rearrange("p (h c) -> p h c", c=HD + 1)
                        nc.vector.memset(vt_r[:, :, HD : HD + 1], 1.0)
                        nc.vector.tensor_copy(
                            vt_r[:, :, 0:HD], ps.rearrange("p (h c) -> p h c", c=HD)
                        )
                        vts.append(vt)

                    # attention heads
                    oT = [
                        po.tile([128, 512], f32, name=f"oT_{l}_{b}_{dk}", tag="o")
                        for dk in range(KT)
                    ]
                    for h in range(H):
                        base = (h % 2) * 64
                        qt = qk[h // 2]
                        kt = qk[4 + h // 2]
                        if l == 0:
                            # Center keys in place (exact - softmax is shift
                            # invariant). Only layer 0 has pre-LN activations
                            # large enough to need it (see SOFT_OFF).
                            kb = pkb.tile([128, 2], f32, name=f"kb_{l}_{b}_{h}", tag="kb")
                            nc.vector.reduce_sum(
                                kb[base : base + 64, 0:1],
                                kt[base : base + 64, :],
                                axis=AX,
                            )
                            nc.scalar.mul(
                                kb[base : base + 64, 1:2],
                                kb[base : base + 64, 0:1],
                                1.0 / S,
                            )
                            nc.vector.tensor_scalar_sub(
                                r(kt[base : base + 64, :]),
                                kt[base : base + 64, :],
                                kb[base : base + 64, 1:2],
                            )
                        eT = []
                        for kti in range(4):
                            ps_s = psc.tile(
                                [128, 512], f32, name=f"ps_s_{l}_{b}_{h}_{kti}", tag="sc"
                            )
                            nc.tensor.matmul(
                                ps_s,
                                r(kt[base : base + 64, kti * 128 : (kti + 1) * 128]),
                                r(qt[base : base + 64, :]),
                                start=True,
                                stop=True,
                            )
                            et = pe_.tile(
                                [128, 512], bf16, name=f"eT_{l}_{b}_{h}_{kti}", tag="e"
                            )
                            nc.scalar.activation(
                                et,
                                ps_s,
                                AF.Exp,
                                bias=col(cols, C_SOFTOFF),
                                scale=1.0 / math.sqrt(HD),
                            )
                            eT.append(et)
                        av = pav.tile([128, 512], f32, name=f"av_{l}_{b}_{h}", tag="av")
                        for kti in range(4):
                            nc.tensor.matmul(
                                av[0 : HD + 1, :],
                                vts[kti][:, h * (HD + 1) : (h + 1) * (HD + 1)],
                                eT[kti],
                                start=(kti == 0),
                                stop=(kti == 3),
                            )
                        rrow = prow.tile([1, 512], f32, name=f"rrow_{l}_{b}_{h}", tag="row")
                        nc.vector.reciprocal(rrow, av[HD : HD + 1, :])
                        rb = pbc.tile([128, 512], f32, name=f"rb_{l}_{b}_{h}", tag="bc")
                        nc.gpsimd.partition_broadcast(rb[0:64, :], rrow)
                        dk = h // 2
                        nc.vector.tensor_tensor(
                            r(oT[dk][base : base + 64, :]), av[0:64, :], rb[0:64, :], OPT.mult
                        )

                    # out_proj + residual + bias
                    yts = []
                    for e in range(KT):
                        ps = pmm.tile([128, 512], f32, name=f"ps_o_{l}_{b}_{e}", tag="mm")
                        for dk in range(KT):
                            nc.tensor.matmul(
                                ps,
                                r(wts["wo"][dk][:, e * 128 : (e + 1) * 128]),
                                r(oT[dk]),
                                start=(dk == 0),
                                stop=(dk == KT - 1),
                            )
                        yt = py.tile([128, 512], f32, name=f"y_{l}_{b}_{e}", tag="y")
                        nc.vector.scalar_tensor_tensor(
                            r(yt), ps, col(cols, C_OUTB + e), x[(e, b)], OPT.add, OPT.add
                        )
                        yts.append(yt)

                    # LN1
                    x1 = _layernorm_fm(
                        nc, tc, mybir, yts, cols, C_LN1G, C_LN1B, l, b, "ln1",
                        px, psq, prow, pbc, pav, r,
                    )
                    for k in range(KT):
                        x1all[(k, b)] = x1[k]

                # FFN + LN2, per batch
                w1n = None
                for b in range(BL):
                    y2 = [None] * KT
                    for chunk in range(2):
                        hts = []
                        for fi in range(8):
                            f = chunk * 8 + fi
                            ps = pmm.tile(
                                [128, 512], f32, name=f"ps_h_{l}_{b}_{f}", tag="mm"
                            )
                            for k in range(KT):
                                nc.tensor.matmul(
                                    ps,
                                    r(wts["w1"][k][:, f * 128 : (f + 1) * 128]),
                                    r(x1all[(k, b)]),
                                    start=(k == 0),
                                    stop=(k == KT - 1),
                                )
                            ht = ph.tile([128, 512], f32, name=f"h_{l}_{b}_{f}", tag="h")
                            nc.scalar.activation(
                                r(ht), ps, AF.Relu, bias=col(cols, C_FFB1 + f), scale=1.0
                            )
                            hts.append(ht)
                        # W2 contraction over this chunk's 8 f-tiles, two
                        # e-tiles at a time (2 PSUM banks); w2 k-tiles are
                        # re-streamed per e-pair (DMA is cheap vs PSUM banks).
                        for ep in range(2):
                            psfs = []
                            for e in (2 * ep, 2 * ep + 1):
                                psf = pff.tile(
                                    [128, 512], f32,
                                    name=f"ps_ff_{l}_{b}_{chunk}_{e}", tag="ff",
                                )
                                psfs.append(psf)
                            for fi in range(8):
                                f = chunk * 8 + fi
                                w2t = pw2.tile(
                                    [128, 256], f32,
                                    name=f"w2_{l}_{b}_{f}_{ep}", tag="w2",
                                )
                                nc.sync.dma_start(
                                    r(w2t),
                                    r(
                                        dw2T.ap()[
                                            l,
                                            f * 128 : (f + 1) * 128,
                                            ep * 256 : (ep + 1) * 256,
                                        ]
                                    ),
                                )
                                for j in range(2):
                                    nc.tensor.matmul(
                                        psfs[j],
                                        r(w2t[:, j * 128 : (j + 1) * 128]),
                                        r(hts[fi]),
                                        start=(fi == 0),
                                        stop=(fi == 7),
                                    )
                            for j in range(2):
                                e = 2 * ep + j
                                psf = psfs[j]
                                if chunk == 0:
                                    yt = py.tile(
                                        [128, 512], f32, name=f"y2_{l}_{b}_{e}", tag="y"
                                    )
                                    nc.vector.scalar_tensor_tensor(
                                        r(yt), psf, col(cols, C_FFB2 + e), x1all[(e, b)],
                                        OPT.add, OPT.add,
                                    )
                                    y2[e] = yt
                                else:
                                    nc.vector.tensor_tensor(r(y2[e]), y2[e], psf, OPT.add)

                    # LN2
                    xn = _layernorm_fm(
                        nc, tc, mybir, y2, cols, C_LN2G, C_LN2B, l, b, "ln2",
                        px, psq, prow, pbc, pav, r,
                    )
                    for k in range(KT):
                        x_next[(k, b)] = xn[k]

                    if b == 0 and l + 1 < L:
                        wts_next = load_layer_weights(l + 1)

                if l + 1 < L:
                    wts = wts_next
                x = x_next

            # ---------------- output head -----------------
            hw1_sb = []
            for k in range(KT):
                t_ = pw1.tile([128, FF], f32, name=f"hw1_{k}", tag="w1")
                nc.sync.dma_start(r(t_), r(dhw1T.ap()[k * 128 : (k + 1) * 128, :]))
                hw1_sb.append(t_)
            hw2c_sb = psmall.tile([128, FT], f32, name="hw2c_sb", tag="small")
            nc.sync.dma_start(r(hw2c_sb), r(dhw2c.ap()))
            hb1c_sb = psmall.tile([128, FT], f32, name="hb1c_sb", tag="small")
            nc.sync.dma_start(hb1c_sb, dhb1c.ap())
            hb2_sb = psmall.tile([1, 1], f32, name="hb2_sb", tag="tiny")
            nc.sync.dma_start(hb2_sb, dhb2.ap())

            last = []
            for k in range(KT):
                lt = psmall.tile([128, BL], f32, name=f"last_{k}", tag="last")
                for b in range(BL):
                    nc.vector.tensor_copy(r(lt[:, b : b + 1]), x[(k, b)][:, S - 1 : S])
                last.append(lt)

            hh = []
            for f in range(FT):
                ps = pmm.tile([128, 512], f32, name=f"ps_hh_{f}", tag="mm")
                for k in range(KT):
                    nc.tensor.matmul(
                        ps[:, 0:BL],
                        r(hw1_sb[k][:, f * 128 : (f + 1) * 128]),
                        r(last[k]),
                        start=(k == 0),
                        stop=(k == KT - 1),
                    )
                ht = psmall.tile([128, BL], f32, name=f"hh_{f}", tag="hh")
                nc.scalar.activation(
                    r(ht), ps[:, 0:BL], AF.Relu, bias=hb1c_sb[:, f : f + 1], scale=1.0
                )
                hh.append(ht)

            ps_out = pav.tile([128, 512], f32, name="ps_out", tag="av")
            for f in range(FT):
                nc.tensor.matmul(
                    ps_out[0:1, 0:BL],
                    r(hw2c_sb[:, f : f + 1]),
                    r(hh[f]),
                    start=(f == 0),
                    stop=(f == FT - 1),
                )
            out_sb = prow.tile([1, 512], f32, name="out_sb", tag="row")
            nc.scalar.activation(
                out_sb[0:1, 0:BL], ps_out[0:1, 0:BL], AF.Identity, bias=hb2_sb, scale=1.0
            )
            nc.sync.dma_start(dout.ap(), out_sb[0:1, 0:BL])

    # Bacc.compile() runs the passes walrus needs: 1-wait-per-instruction
    # splitting, gpsimd library loads, ISA byte encoding, act table loads.
    nc.compile()
    return nc


def _layernorm_fm(nc, tc, mybir, yts, cols, cg, cb, l, b, name, px, psq, prow, pbc, pav, r):
    """Feature-major layernorm over partition dim of 4 x [128, 512] tiles.

    Returns 4 normalized tiles (from pool px, tag 'x').
    """
    OPT = mybir.AluOpType
    AF = mybir.ActivationFunctionType
    ones = cols[:, C_ONES : C_ONES + 1]

    ps_sum = pav.tile([128, 512], mybir.dt.float32, name=f"ps_sum_{name}_{l}_{b}", tag="av")
    for k in range(KT):
        nc.tensor.matmul(
            ps_sum[0:1, :], r(ones), r(yts[k]), start=(k == 0), stop=(k == KT - 1)
        )
    mrow = prow.tile([1, 512], mybir.dt.float32, name=f"m_{name}_{l}_{b}", tag="row")
    nc.scalar.mul(mrow, ps_sum[0:1, :], 1.0 / D)

    sqs = []
    for k in range(KT):
        sq = psq.tile([128, 512], mybir.dt.float32, name=f"sq_{name}_{l}_{b}_{k}", tag="sq")
        nc.scalar.activation(r(sq), yts[k], AF.Square)
        sqs.append(sq)
    ps_sq = pav.tile([128, 512], mybir.dt.float32, name=f"ps_sq_{name}_{l}_{b}", tag="av")
    for k in range(KT):
        nc.tensor.matmul(
            ps_sq[0:1, :], r(ones), r(sqs[k]), start=(k == 0), stop=(k == KT - 1)
        )
    vrow = prow.tile([1, 512], mybir.dt.float32, name=f"v_{name}_{l}_{b}", tag="row")
    nc.scalar.mul(vrow, ps_sq[0:1, :], 1.0 / D)
    msq = prow.tile([1, 512], mybir.dt.float32, name=f"msq_{name}_{l}_{b}", tag="row")
    nc.scalar.activation(msq, mrow, AF.Square)
    nc.vector.tensor_tensor(vrow, vrow, msq, OPT.subtract)  # var = E[y^2] - m^2
    nc.vector.tensor_scalar_add(vrow, vrow, EPS)
    nc.scalar.activation(msq, vrow, AF.Sqrt)  # std (reuse msq tile)
    nc.vector.reciprocal(vrow, msq)  # rstd (reuse vrow tile)

    mb = pbc.tile([128, 512], mybir.dt.float32, name=f"mb_{name}_{l}_{b}", tag="bc")
    nc.gpsimd.partition_broadcast(mb, mrow)
    rstdb = pbc.tile([128, 512], mybir.dt.float32, name=f"rstdb_{name}_{l}_{b}", tag="bc")
    nc.gpsimd.partition_broadcast(rstdb, vrow)

    out = []
    for k in range(KT):
        nc.vector.tensor_tensor(r(yts[k]), yts[k], mb, OPT.subtract)
        xn = px.tile([128, 512], mybir.dt.float32, name=f"xn_{name}_{l}_{b}_{k}", tag="x")
        nc.vector.scalar_tensor_tensor(
            r(xn), yts[k], cols[:, cg + k : cg + k + 1], rstdb, OPT.mult, OPT.mult
        )
        out.append(xn)
    return out


def host_prep(inputs):
    """Transpose/pack weights on host; build per-core input maps."""
    ins = {k: np.asarray(v, dtype=np.float32) for k, v in inputs.items()}

    wqkvT = np.ascontiguousarray(ins["in_proj_w"].transpose(0, 2, 1))  # [L, D, 3D]
    woT = np.ascontiguousarray(ins["out_w"].transpose(0, 2, 1))  # [L, D, D]
    w1T = np.ascontiguousarray(ins["ff_w1"].transpose(0, 2, 1))  # [L, D, FF]
    w2T = np.ascontiguousarray(ins["ff_w2"].transpose(0, 2, 1))  # [L, FF, D]

    cols = np.zeros((L, 128, 64), dtype=np.float32)
    for l in range(L):
        for m in range(12):
            cols[l, :, C_QKVB + m] = ins["in_proj_b"][l, m * 128 : (m + 1) * 128]
        for e in range(4):
            cols[l, :, C_OUTB + e] = ins["out_b"][l, e * 128 : (e + 1) * 128]
        for f in range(FT):
            cols[l, :, C_FFB1 + f] = ins["ff_b1"][l, f * 128 : (f + 1) * 128]
        for e in range(4):
            cols[l, :, C_FFB2 + e] = ins["ff_b2"][l, e * 128 : (e + 1) * 128]
        for k in range(4):
            cols[l, :, C_LN1G + k] = ins["ln1_g"][l, k * 128 : (k + 1) * 128]
            cols[l, :, C_LN1B + k] = ins["ln1_b"][l, k * 128 : (k + 1) * 128]
            cols[l, :, C_LN2G + k] = ins["ln2_g"][l, k * 128 : (k + 1) * 128]
            cols[l, :, C_LN2B + k] = ins["ln2_b"][l, k * 128 : (k + 1) * 128]
        cols[l, :, C_ONES] = 1.0
        cols[l, :, C_SOFTOFF] = -np.float32(SOFT_OFF[l])
        for m in range(4):
            cols[l, :, C_INPB + m] = (
                ins["inp_b"][m * 128 : (m + 1) * 128] * np.float32(SQRT_D)
            )

    hw1T = np.ascontiguousarray(ins["op_w1"].T)  # [D, FF]
    hw2c = np.zeros((128, FT), dtype=np.float32)
    hb1c = np.zeros((128, FT), dtype=np.float32)
    for f in range(FT):
        hw2c[:, f] = ins["op_w2"][0, f * 128 : (f + 1) * 128]
        hb1c[:, f] = ins["op_b1"][f * 128 : (f + 1) * 128]
    hb2 = ins["op_b2"].reshape(1, 1)

    peT = _pos_encoding_T()

    shared = dict(
        inpwT=np.ascontiguousarray(ins["inp_w"].T),
        peT=peT,
        wqkvT=wqkvT,
        woT=woT,
        w1T=w1T,
        w2T=w2T,
        cols=cols,
        hw1T=hw1T,
        hw2c=hw2c,
        hb1c=hb1c,
        hb2=hb2,
    )
    in_maps = []
    for c in range(NCORES):
        m = dict(shared)
        shard = ins["src"][c * BL : (c + 1) * BL]  # [BL, S, IN]
        m["srcT"] = np.ascontiguousarray(shard.reshape(T, IN).T)
        in_maps.append(m)
    return in_maps


_NC = None


def _get_nc():
    global _NC
    if _NC is None:
        _NC = build_program()
    return _NC


def run(inputs, **kwargs):
    from concourse import bass_utils

    nc = _get_nc()
    in_maps = host_prep(inputs)
    res = bass_utils.run_bass_kernel_spmd(nc, in_maps, core_ids=list(range(NCORES)), **kwargs)
    out = np.concatenate([res.results[c]["out"] for c in range(NCORES)], axis=0)
    return out.astype(np.float32), res


def kernel(**inputs):
    out, _ = run(inputs)
    return out
